# revision 34
# baseline (speedup 1.0000x reference)
"""Llama attention layer (B=2, S=2048, H=4096, 32 heads, fp32 io) on 8 trn2 cores.

Sharding: tensor-parallel over heads. Each core owns 4 heads: W_qkv column
shard [4096, 3*512] (bf16), W_o row shard [512, 4096] (bf16). Each core
computes qkv proj + RoPE + causal attention for its heads + its o_proj
partial; the host sums the 8 partials (the "all-reduce") and untransposes
the output (kernel emits o_partial^T in bf16).

v3 vs v2 (overlap restructure, from trace: PE busy was 88.3%, with the
b1-pairs+p3 region 70us over floor from shallow p3 psum groups whose
evacuations queued behind pair work in the strict-FIFO engines):
  - attention pairs run as 2-wide interleaved "flights": pair A's exp/mask
    chain latency is hidden behind pair B's score/PV matmuls, so the PE
    self-fills during attention and the p3/p1 filler is only a top-up.
  - pair prefetches are emitted in 512-token chunks, so transposes fire
    progressively as the producing phase-1 blocks complete (pair starts
    and batch boundaries no longer expose the ~15us transpose latency).
  - softmax row-sums accumulate in fp16 (2x DVE 16-bit mode); the gpsimd
    partition-reduce upcasts to fp32 for the reciprocal.
  - o_proj partials are written bf16 (halves the 67MB output DMA).
  - p3 tail rotates its psum tiles across 3 pools (6 banks) so matmul
    groups never wait on evacuation.
  - first hblk/wch DMAs split per-kc so the first matmul starts sooner.
"""

import numpy as np
import ml_dtypes

import concourse.bass as bass
import concourse.tile as tile
from concourse import bacc, mybir
from concourse.bass_isa import ReduceOp
from concourse.bass_utils import run_bass_kernel_spmd

# ---- problem constants (hardcoded per contract) ----
HIDDEN = 4096
NH = 32
D = 128
B = 2
S = 2048
TOK = B * S            # 4096 tokens
N_CORES = 8
HC = NH // N_CORES     # 4 heads per core
FH = HC * D            # 512 features per core for each of q/k/v
SCALING = D ** -0.5
ROPE_BASE = 10000.0

BF16 = mybir.dt.bfloat16
F16 = mybir.dt.float16
F32 = mybir.dt.float32

TBLK = 512             # tokens per phase-1 block
NTB = TOK // TBLK      # 8
QT = 512               # q columns per phase-2 tile
NQT = S // QT          # 4
NKC = S // 128         # 16 k chunks per sequence
NOB = HIDDEN // 128    # 32 output-column chunks in phase 3
EXP = mybir.ActivationFunctionType.Exp


class Filler:
    """Pulls emission units (generators yielding ~tensor-ns) on demand."""

    def __init__(self, gens):
        self.gens = list(gens)

    def pull(self, ns):
        while ns > 0 and self.gens:
            try:
                ns -= next(self.gens[0])
            except StopIteration:
                self.gens.pop(0)

    def drain(self):
        self.pull(float("inf"))


def build_nc():
    nc = bacc.Bacc("TRN2", target_bir_lowering=False, debug=False,
                   num_devices=N_CORES)
    hT = nc.dram_tensor("hT", [NTB, 4, 128, 8 * TBLK], BF16, kind="ExternalInput").ap()
    w = nc.dram_tensor("w", [3, 4, 128, 8 * FH], BF16, kind="ExternalInput").ap()
    wo = nc.dram_tensor("wo", [NOB, 128, HC, 128], BF16, kind="ExternalInput").ap()
    csn = nc.dram_tensor("csn", [TOK, 2, FH], BF16, kind="ExternalInput").ap()
    msk = nc.dram_tensor("msk", [128, 2, 2, QT], F16, kind="ExternalInput").ap()
    outp = nc.dram_tensor("outp", [HIDDEN, TOK], BF16, kind="ExternalOutput").ap()

    with tile.TileContext(nc) as tc:
        from contextlib import ExitStack
        with ExitStack() as ctx:
            # PSUM: ps 2 banks + ps2 4 banks + pv 2 banks = 8
            psp = ctx.enter_context(tc.tile_pool(name="ps", bufs=2, space="PSUM"))
            ps2p = ctx.enter_context(tc.tile_pool(name="ps2", bufs=2, space="PSUM"))
            pvp = ctx.enter_context(tc.tile_pool(name="pv", bufs=2, space="PSUM"))
            gsb = ctx.enter_context(tc.tile_pool(name="gsb", bufs=1))
            pairp = ctx.enter_context(tc.tile_pool(name="pair", bufs=6))
            ptp = ctx.enter_context(tc.tile_pool(name="pt", bufs=6))
            accp = ctx.enter_context(tc.tile_pool(name="acc", bufs=4))
            accfp = ctx.enter_context(tc.tile_pool(name="accf", bufs=4))
            lnvp = ctx.enter_context(tc.tile_pool(name="lnv", bufs=2))
            aup = ctx.enter_context(tc.tile_pool(name="au", bufs=4))
            dscr = ctx.enter_context(tc.tile_pool(name="dscr", bufs=1, space="DRAM"))

            # per-block scratch tiles: DRAM deps are tile-granular, so pair
            # transposes of block c can fire as soon as block c's projection
            # lands instead of waiting for the whole batch
            scr = [[dscr.tile([TBLK, FH], BF16, tag=f"s{j3}_{T}",
                              name=f"scr{j3}_{T}") for T in range(NTB)]
                   for j3 in range(3)]

            attn_s = [gsb.tile([128, HC, S], BF16, tag="attn0", name="attn0"),
                      gsb.tile([128, HC, S], BF16, tag="attn1", name="attn1")]
            mskt = gsb.tile([128, 2, 2, QT], F16, tag="msk", name="mskt")
            nc.sync.dma_start(out=mskt, in_=msk)

            # ---------------- phase 2: interleaved pair flights ---------------
            def prefetch_flight(b, hhs, pool):
                # chunked so each transpose fires as its phase-1 block lands;
                # issued from the SCALAR queue: the sync queue is an in-order
                # pipe paced by phase-1's pool-slot waits, so transposes
                # issued there can't start until the whole drain clears (and
                # they'd head-of-line block later fill DMAs). Scalar is idle
                # outside the exp bursts.
                tiles = []
                for hh in hhs:
                    qTp = pool.tile([128, S], BF16, tag="pair", name="qTp")
                    kTp = pool.tile([128, S], BF16, tag="pair", name="kTp")
                    vp = pool.tile([128, NKC, D], BF16, tag="pair", name="vp")
                    tiles.append((qTp, kTp, vp))
                for c in range(4):
                    blk = b * 4 + c
                    for hh, (qTp, kTp, vp) in zip(hhs, tiles):
                        cols = slice(hh * D, (hh + 1) * D)
                        nc.sync.dma_start_transpose(
                            out=qTp[:, c * TBLK:(c + 1) * TBLK],
                            in_=scr[0][blk][:, cols])
                        nc.sync.dma_start_transpose(
                            out=kTp[:, c * TBLK:(c + 1) * TBLK],
                            in_=scr[1][blk][:, cols])
                        nc.sync.dma_start(
                            out=vp[:, 4 * c:4 * (c + 1), :],
                            in_=scr[2][blk][:, cols].rearrange(
                                "(kc p) d -> p kc d", p=128))
                return tiles

            pending_norm = []

            def flush_norm(keep):
                # deferred reciprocal+normalize: by flush time the gpsimd
                # all_reduce is long done, so Vector never blocks on it
                while len(pending_norm) > keep:
                    pending_norm.pop(0)()

            def mk_norm(accf_, attn_u_, b_, hh_, qt_):
                def emit():
                    linv = lnvp.tile([128, QT], F32, tag="lnv", name="linv")
                    nc.vector.reciprocal_approx_fast(out=linv, in_=accf_)
                    nc.vector.tensor_mul(
                        attn_s[b_][:, hh_, qt_ * QT:(qt_ + 1) * QT],
                        attn_u_, linv)
                return emit

            def run_flight(b, hh2, tiles2, filler, pull_ns, skip_fill_groups=0):
                npair = len(hh2)
                prev = [None] * npair
                pvs = [None] * npair
                accs = [None] * npair
                gcnt = 0
                for qt in range(NQT):
                    flush_norm(keep=npair)
                    nkc = 4 * (qt + 1)
                    for X in range(npair):
                        pvs[X] = pvp.tile([128, QT], F32, tag="pv", name="pv")
                        accs[X] = accp.tile([128, QT], F16, tag="acc", name="acc")
                    for g in range(nkc // 2):
                        diag = g >= 2 * qt
                        rs = [max(0, 128 * (2 * g + s2 - 4 * qt)) for s2 in (0, 1)]
                        psels = [None] * npair
                        for X in range(npair):
                            qTp, kTp, _vp = tiles2[X]
                            acc = accs[X]
                            sc = ps2p.tile([128, 2, QT], F32, tag="ps2", name="sc")
                            for s2 in range(2):
                                kc = 2 * g + s2
                                nc.tensor.matmul(
                                    sc[:, s2, rs[s2]:],
                                    lhsT=kTp[:, kc * 128:(kc + 1) * 128],
                                    rhs=qTp[:, qt * QT + rs[s2]:(qt + 1) * QT],
                                    start=True, stop=True)
                            pt2 = ptp.tile([128, 2, QT], F16, tag="pt", name="pt2")
                            if diag:
                                for s2 in range(2):
                                    r = rs[s2]
                                    nc.scalar.activation(
                                        out=pt2[:, s2, r:], in_=sc[:, s2, r:],
                                        func=EXP, scale=SCALING)
                                psel = ptp.tile([128, 2, QT], F16, tag="pt",
                                                name="ptm")
                                for s2 in range(2):
                                    r = rs[s2]
                                    nc.vector.tensor_mul(
                                        psel[:, s2, r:], pt2[:, s2, r:],
                                        mskt[:, g - 2 * qt, s2, r:])
                            else:
                                nc.scalar.activation(out=pt2, in_=sc, func=EXP,
                                                     scale=SCALING)
                                psel = pt2
                            # softmax denominator accumulation (Vector, fp16 2x)
                            if g == 0:
                                if diag:  # qt == 0: rs == [0, 128]
                                    nc.vector.tensor_copy(out=acc, in_=psel[:, 0, :])
                                    nc.vector.tensor_add(
                                        acc[:, 128:], acc[:, 128:], psel[:, 1, 128:])
                                else:
                                    nc.vector.tensor_add(
                                        acc, psel[:, 0, :], psel[:, 1, :])
                            else:
                                for s2 in range(2):
                                    r = rs[s2]
                                    nc.vector.tensor_add(
                                        acc[:, r:], acc[:, r:], psel[:, s2, r:])
                            psels[X] = psel
                        gcnt += 1
                        if gcnt > skip_fill_groups:
                            filler.pull(pull_ns)
                        for X in range(npair):
                            if prev[X] is not None:
                                prev[X]()

                            def mk_pv(psel_, pv_, vp_, g_, rs_, nkc_):
                                def emit():
                                    for s2 in range(2):
                                        kc = 2 * g_ + s2
                                        nc.tensor.matmul(
                                            pv_[:, rs_[s2]:], lhsT=vp_[:, kc, :],
                                            rhs=psel_[:, s2, rs_[s2]:],
                                            start=(kc == 0), stop=(kc == nkc_ - 1))
                                return emit
                            prev[X] = mk_pv(psels[X], pvs[X], tiles2[X][2],
                                            g, rs, nkc)
                    for X in range(npair):
                        prev[X]()
                        prev[X] = None
                        # stage unnormalized attn to SBUF so the pv psum slot
                        # is released without waiting on the l-reduction chain
                        attn_u = aup.tile([128, QT], BF16, tag="au", name="attn_u")
                        nc.vector.tensor_copy(out=attn_u, in_=pvs[X])
                        accf = accfp.tile([128, QT], F32, tag="accf", name="accf")
                        nc.gpsimd.partition_all_reduce(
                            accf, accs[X], 128, ReduceOp.add)
                        pending_norm.append(mk_norm(accf, attn_u, b, hh2[X], qt))

            # ---------------- phase 1: qkv projections + rope ----------------
            def p1_stream(blocks, pools, cold_start=False):
                hp, wp, csp, rtp, stp = pools

                def load_w(j3, i, fine=False):
                    t_ = wp.tile([128, 8, FH], BF16, tag="wch", name="wch")
                    src = w[j3, i].rearrange("p (kc f) -> p kc f", kc=8)
                    if fine:
                        for kq in range(4):
                            nc.sync.dma_start(
                                out=t_[:, 2 * kq:2 * kq + 2, :],
                                in_=src[:, 2 * kq:2 * kq + 2, :])
                    else:
                        nc.sync.dma_start(out=t_, in_=src)
                    return t_

                for T in blocks:
                    first = cold_start and T == blocks[0]
                    hblk = []
                    first_w = []
                    for i in range(4):
                        t_ = hp.tile([128, 8, TBLK], BF16, tag="hblk", name="hblk")
                        src = hT[T, i].rearrange("p (kc t) -> p kc t", kc=8)
                        if first:
                            # cold start: interleave h and w(j3=0) tile loads so
                            # the first matmul's operands aren't queued behind
                            # 4MB of hidden-state packets
                            for kq in range(4):
                                nc.sync.dma_start(
                                    out=t_[:, 2 * kq:2 * kq + 2, :],
                                    in_=src[:, 2 * kq:2 * kq + 2, :])
                            first_w.append(load_w(0, i, fine=(i == 0)))
                        else:
                            nc.sync.dma_start(out=t_, in_=src)
                        hblk.append(t_)
                    csts = []
                    for tt in range(4):
                        cst = csp.tile([128, 2, HC, D], BF16, tag="cs", name="cst")
                        r0 = T * TBLK + tt * 128
                        nc.sync.dma_start(
                            out=cst,
                            in_=csn[r0:r0 + 128].rearrange("p c (h d) -> p c h d", h=HC))
                        csts.append(cst)
                    yield 0
                    for j3 in range(3):
                        if first and j3 == 0:
                            wch = first_w
                        else:
                            wch = [load_w(j3, i) for i in range(4)]
                        yield 0
                        for tt in range(4):
                            ps = psp.tile([128, HC, D], F32, tag="ps", name="ps")
                            for half in range(8):
                                i = half // 2
                                for kc in range(4 * (half % 2), 4 * (half % 2) + 4):
                                    nc.tensor.matmul(
                                        ps,
                                        lhsT=hblk[i][:, kc, tt * 128:(tt + 1) * 128],
                                        rhs=wch[i][:, kc, :],
                                        start=(half == 0 and kc == 0),
                                        stop=(half == 7 and kc == 7),
                                    )
                                yield 852
                            st = stp.tile([128, HC, D], BF16, tag="stage", name="st")
                            if j3 < 2:
                                cst = csts[tt]
                                half_d = D // 2
                                tr = rtp.tile([128, HC, D], F32, tag="rtmp", name="tr")
                                tcos = rtp.tile([128, HC, D], F32, tag="rtmp", name="tcos")
                                nc.vector.tensor_mul(
                                    tr[:, :, 0:half_d], ps[:, :, half_d:D],
                                    cst[:, 1, :, 0:half_d])
                                nc.vector.tensor_mul(
                                    tr[:, :, half_d:D], ps[:, :, 0:half_d],
                                    cst[:, 1, :, half_d:D])
                                nc.vector.tensor_mul(tcos, ps, cst[:, 0])
                                nc.vector.tensor_add(st, tr, tcos)
                            else:
                                nc.vector.tensor_copy(out=st, in_=ps)
                            r0 = tt * 128
                            nc.sync.dma_start(
                                out=scr[j3][T][r0:r0 + 128, :], in_=st)
                            yield 0

            # ---------------- phase 3: o_proj partial (transposed out) -------
            def p3_stream(tbs, pools, pso_srcs, evac="alt"):
                wop, ostp = pools
                cnt = 0
                for ob in range(NOB):
                    wot = wop.tile([128, HC, 128], BF16, tag="wo", name="wot")
                    nc.sync.dma_start(out=wot, in_=wo[ob])
                    yield 0
                    for tb in tbs:
                        pool, tag = pso_srcs[cnt % len(pso_srcs)]
                        pso = pool.tile([128, TBLK], F32, tag=tag, name="pso")
                        for kc in range(HC):
                            nc.tensor.matmul(
                                pso, lhsT=wot[:, kc, :],
                                rhs=attn_s[tb // 4][:, kc,
                                                    (tb % 4) * TBLK:(tb % 4 + 1) * TBLK],
                                start=(kc == 0), stop=(kc == HC - 1))
                        yield 852
                        ot = ostp.tile([128, TBLK], BF16, tag="ost", name="ot")
                        # keep scalar free for pair exps when evac="vector"
                        if evac == "vector" or cnt % 2 == 0:
                            nc.vector.tensor_copy(out=ot, in_=pso)
                        else:
                            nc.scalar.copy(out=ot, in_=pso)
                        nc.sync.dma_start(
                            out=outp[ob * 128:(ob + 1) * 128,
                                     tb * TBLK:(tb + 1) * TBLK], in_=ot)
                        cnt += 1
                        yield 0

            # ---------------- schedule -------------------------------------
            with ExitStack() as p1ctx:
                p1pools = (
                    p1ctx.enter_context(tc.tile_pool(name="hblk", bufs=6)),
                    p1ctx.enter_context(tc.tile_pool(name="wch", bufs=6)),
                    p1ctx.enter_context(tc.tile_pool(name="cs", bufs=5)),
                    p1ctx.enter_context(tc.tile_pool(name="rtmp", bufs=2)),
                    p1ctx.enter_context(tc.tile_pool(name="stage", bufs=4)),
                )
                Filler([p1_stream(range(4), p1pools, cold_start=True)]).drain()
                fb = Filler([p1_stream(range(4, NTB), p1pools)])
                t00 = prefetch_flight(0, (0,), pairp)[0]
                t01 = prefetch_flight(0, (1,), pairp)[0]
                # no fill in the first 6 groups: p1B's first DMAs are ~15us
                # out, a stalled fill unit would block the in-order PE
                run_flight(0, (0,), (t00,), fb, pull_ns=700, skip_fill_groups=6)
                t02 = prefetch_flight(0, (2,), pairp)[0]
                run_flight(0, (1,), (t01,), fb, pull_ns=700)
                t03 = prefetch_flight(0, (3,), pairp)[0]
                run_flight(0, (2,), (t02,), fb, pull_ns=700)
                run_flight(0, (3,), (t03,), fb, pull_ns=700)
                flush_norm(keep=0)
                t10 = prefetch_flight(1, (0,), pairp)[0]
                fb.drain()

            with ExitStack() as p3ctx:
                p3pools = (
                    p3ctx.enter_context(tc.tile_pool(name="wop", bufs=3)),
                    p3ctx.enter_context(tc.tile_pool(name="ost", bufs=6)),
                )
                # fresh slots for pairs (1,2)/(1,3): their transposes issue
                # the moment the sync queue reaches them instead of blocking
                # it (and the fill DMAs behind) on a pairp slot release that
                # is execution-gated on the previous pair finishing
                pairb = p3ctx.enter_context(tc.tile_pool(name="pairb", bufs=6))
                fc = Filler([p3_stream(range(4), p3pools, [(psp, "ps")],
                                       evac="alt")])
                fc.pull(12000)
                t11 = prefetch_flight(1, (1,), pairp)[0]
                run_flight(1, (0,), (t10,), fc, pull_ns=700)
                t12 = prefetch_flight(1, (2,), pairb)[0]
                run_flight(1, (1,), (t11,), fc, pull_ns=700)
                t13 = prefetch_flight(1, (3,), pairb)[0]
                run_flight(1, (2,), (t12,), fc, pull_ns=700)
                run_flight(1, (3,), (t13,), fc, pull_ns=700)
                flush_norm(keep=0)
                fc.drain()
                Filler([p3_stream(range(4, NTB), p3pools,
                                  [(psp, "ps"), (pvp, "pv"),
                                   (ps2p, "ps2")])]).drain()

    nc.compile()
    return nc


_NC_CACHE = {}


def get_nc():
    if "nc" not in _NC_CACHE:
        _NC_CACHE["nc"] = build_nc()
    return _NC_CACHE["nc"]


def prep_in_maps(positions, hidden_states, W_qkv, W_o):
    """Host-side sharding + layout prep. Returns per-core input maps."""
    bf16 = ml_dtypes.bfloat16
    hid = np.asarray(hidden_states, np.float32).reshape(TOK, HIDDEN)
    # hT[T, i, p, kc, t] = hid[T*512+t, i*1024+kc*128+p]
    hT = np.ascontiguousarray(
        hid.reshape(NTB, TBLK, 4, 8, 128).transpose(0, 2, 4, 3, 1)
    ).reshape(NTB, 4, 128, 8 * TBLK).astype(bf16)

    pos = np.asarray(positions).reshape(TOK).astype(np.float32)
    half = D // 2
    inv = ROPE_BASE ** (-np.arange(half, dtype=np.float32) / half)
    ang = pos[:, None] * inv[None, :]                      # [TOK, 64]
    cos = np.cos(ang)
    sin = np.sin(ang)
    cos128 = np.concatenate([cos, cos], axis=1)            # [TOK, 128]
    sin128 = np.concatenate([-sin, sin], axis=1)
    csn = np.empty((TOK, 2, FH), np.float32)
    csn[:, 0, :] = np.tile(cos128, HC)
    csn[:, 1, :] = np.tile(sin128, HC)
    csn = csn.astype(bf16)

    kk = np.arange(128)[:, None]
    qq = np.arange(QT)[None, :]
    msk = np.stack([(qq >= kk + o * 128) for o in range(4)], axis=1)
    msk = msk.reshape(128, 2, 2, QT).astype(np.float16)     # [128, 2, 2, 512]

    Wq = np.asarray(W_qkv, np.float32)
    Wo = np.asarray(W_o, np.float32)
    in_maps = []
    for c in range(N_CORES):
        wc = np.concatenate(
            [Wq[:, q0 * HIDDEN + c * FH: q0 * HIDDEN + (c + 1) * FH]
             for q0 in range(3)], axis=1)                   # [4096, 1536]
        # w[j3, i, p, kc, f] = wc[i*1024+kc*128+p, j3*512+f]
        wcp = np.ascontiguousarray(
            wc.reshape(4, 8, 128, 3, FH).transpose(3, 0, 2, 1, 4)
        ).reshape(3, 4, 128, 8 * FH).astype(bf16)
        woc = Wo[c * FH:(c + 1) * FH, :]                    # [512, 4096]
        # wo[ob, p, kc, o] = woc[kc*128+p, ob*128+o]
        wop = np.ascontiguousarray(
            woc.reshape(HC, 128, NOB, 128).transpose(2, 1, 0, 3)
        ).astype(bf16)
        in_maps.append({"hT": hT, "w": wcp, "wo": wop, "csn": csn, "msk": msk})
    return in_maps


def combine_outputs(outps):
    """Sum per-core bf16 o_partial^T [HIDDEN, TOK] and untranspose."""
    out = np.zeros((HIDDEN, TOK), np.float32)
    for o in outps:
        out += np.asarray(o).astype(np.float32)
    return np.ascontiguousarray(out.T).astype(np.float32).reshape(B, S, HIDDEN)


def kernel(positions, hidden_states, W_qkv, W_o):
    nc = get_nc()
    in_maps = prep_in_maps(positions, hidden_states, W_qkv, W_o)
    res = run_bass_kernel_spmd(nc, in_maps, list(range(N_CORES)))
    return combine_outputs([res.results[c]["outp"] for c in range(N_CORES)])


# revision 37
# speedup vs baseline: 1.0129x; 1.0129x over previous
"""Llama attention layer (B=2, S=2048, H=4096, 32 heads, fp32 io) on 8 trn2 cores.

Sharding: tensor-parallel over heads. Each core owns 4 heads: W_qkv column
shard [4096, 3*512] (bf16), W_o row shard [512, 4096] (bf16). Each core
computes qkv proj + RoPE + causal attention for its heads + its o_proj
partial; the host sums the 8 partials (the "all-reduce") and untransposes
the output (kernel emits o_partial^T in bf16).

v3 vs v2 (overlap restructure, from trace: PE busy was 88.3%, with the
b1-pairs+p3 region 70us over floor from shallow p3 psum groups whose
evacuations queued behind pair work in the strict-FIFO engines):
  - attention pairs run as 2-wide interleaved "flights": pair A's exp/mask
    chain latency is hidden behind pair B's score/PV matmuls, so the PE
    self-fills during attention and the p3/p1 filler is only a top-up.
  - pair prefetches are emitted in 512-token chunks, so transposes fire
    progressively as the producing phase-1 blocks complete (pair starts
    and batch boundaries no longer expose the ~15us transpose latency).
  - softmax row-sums accumulate in fp16 (2x DVE 16-bit mode); the gpsimd
    partition-reduce upcasts to fp32 for the reciprocal.
  - o_proj partials are written bf16 (halves the 67MB output DMA).
  - p3 tail rotates its psum tiles across 3 pools (6 banks) so matmul
    groups never wait on evacuation.
  - first hblk/wch DMAs split per-kc so the first matmul starts sooner.
"""

import numpy as np
import ml_dtypes

import concourse.bass as bass
import concourse.tile as tile
from concourse import bacc, mybir
from concourse.bass_isa import ReduceOp
from concourse.bass_utils import run_bass_kernel_spmd

# ---- problem constants (hardcoded per contract) ----
HIDDEN = 4096
NH = 32
D = 128
B = 2
S = 2048
TOK = B * S            # 4096 tokens
N_CORES = 8
HC = NH // N_CORES     # 4 heads per core
FH = HC * D            # 512 features per core for each of q/k/v
SCALING = D ** -0.5
ROPE_BASE = 10000.0

BF16 = mybir.dt.bfloat16
F16 = mybir.dt.float16
F32 = mybir.dt.float32

TBLK = 512             # tokens per phase-1 block
NTB = TOK // TBLK      # 8
QT = 512               # q columns per phase-2 tile
NQT = S // QT          # 4
NKC = S // 128         # 16 k chunks per sequence
NOB = HIDDEN // 128    # 32 output-column chunks in phase 3
EXP = mybir.ActivationFunctionType.Exp


class Filler:
    """Pulls emission units (generators yielding ~tensor-ns) on demand."""

    def __init__(self, gens):
        self.gens = list(gens)

    def pull(self, ns):
        while ns > 0 and self.gens:
            try:
                ns -= next(self.gens[0])
            except StopIteration:
                self.gens.pop(0)

    def drain(self):
        self.pull(float("inf"))


def build_nc():
    nc = bacc.Bacc("TRN2", target_bir_lowering=False, debug=False,
                   num_devices=N_CORES)
    hT = nc.dram_tensor("hT", [NTB, 4, 128, 8 * TBLK], BF16, kind="ExternalInput").ap()
    w = nc.dram_tensor("w", [3, 4, 128, 8 * FH], BF16, kind="ExternalInput").ap()
    wo = nc.dram_tensor("wo", [NOB, 128, HC, 128], BF16, kind="ExternalInput").ap()
    csn = nc.dram_tensor("csn", [TOK, 2, FH], BF16, kind="ExternalInput").ap()
    msk = nc.dram_tensor("msk", [128, 2, 2, QT], F16, kind="ExternalInput").ap()
    outp = nc.dram_tensor("outp", [HIDDEN, TOK], BF16, kind="ExternalOutput").ap()

    with tile.TileContext(nc) as tc:
        from contextlib import ExitStack
        with ExitStack() as ctx:
            # PSUM: ps 2 banks + ps2 4 banks + pv 2 banks = 8
            psp = ctx.enter_context(tc.tile_pool(name="ps", bufs=2, space="PSUM"))
            ps2p = ctx.enter_context(tc.tile_pool(name="ps2", bufs=2, space="PSUM"))
            pvp = ctx.enter_context(tc.tile_pool(name="pv", bufs=2, space="PSUM"))
            gsb = ctx.enter_context(tc.tile_pool(name="gsb", bufs=1))
            pairp = ctx.enter_context(tc.tile_pool(name="pair", bufs=6))
            ptp = ctx.enter_context(tc.tile_pool(name="pt", bufs=6))
            accp = ctx.enter_context(tc.tile_pool(name="acc", bufs=4))
            accfp = ctx.enter_context(tc.tile_pool(name="accf", bufs=4))
            lnvp = ctx.enter_context(tc.tile_pool(name="lnv", bufs=2))
            aup = ctx.enter_context(tc.tile_pool(name="au", bufs=4))
            dscr = ctx.enter_context(tc.tile_pool(name="dscr", bufs=1, space="DRAM"))

            # per-block scratch tiles: DRAM deps are tile-granular, so pair
            # transposes of block c can fire as soon as block c's projection
            # lands instead of waiting for the whole batch
            scr = [[dscr.tile([TBLK, FH], BF16, tag=f"s{j3}_{T}",
                              name=f"scr{j3}_{T}") for T in range(NTB)]
                   for j3 in range(3)]

            attn_s = [gsb.tile([128, HC, S], BF16, tag="attn0", name="attn0"),
                      gsb.tile([128, HC, S], BF16, tag="attn1", name="attn1")]
            mskt = gsb.tile([128, 2, 2, QT], F16, tag="msk", name="mskt")
            nc.sync.dma_start(out=mskt, in_=msk)

            # ---------------- phase 2: interleaved pair flights ---------------
            def prefetch_flight(b, hhs, pool):
                # chunked so each transpose fires as its phase-1 block lands;
                # issued from the SCALAR queue: the sync queue is an in-order
                # pipe paced by phase-1's pool-slot waits, so transposes
                # issued there can't start until the whole drain clears (and
                # they'd head-of-line block later fill DMAs). Scalar is idle
                # outside the exp bursts.
                tiles = []
                for hh in hhs:
                    qTp = pool.tile([128, S], BF16, tag="pair", name="qTp")
                    kTp = pool.tile([128, S], BF16, tag="pair", name="kTp")
                    vp = pool.tile([128, NKC, D], BF16, tag="pair", name="vp")
                    tiles.append((qTp, kTp, vp))
                for c in range(4):
                    blk = b * 4 + c
                    for hh, (qTp, kTp, vp) in zip(hhs, tiles):
                        cols = slice(hh * D, (hh + 1) * D)
                        nc.sync.dma_start_transpose(
                            out=qTp[:, c * TBLK:(c + 1) * TBLK],
                            in_=scr[0][blk][:, cols])
                        nc.sync.dma_start_transpose(
                            out=kTp[:, c * TBLK:(c + 1) * TBLK],
                            in_=scr[1][blk][:, cols])
                        nc.sync.dma_start(
                            out=vp[:, 4 * c:4 * (c + 1), :],
                            in_=scr[2][blk][:, cols].rearrange(
                                "(kc p) d -> p kc d", p=128))
                return tiles

            pending_norm = []

            def flush_norm(keep):
                # deferred reciprocal+normalize: by flush time the gpsimd
                # all_reduce is long done, so Vector never blocks on it
                while len(pending_norm) > keep:
                    pending_norm.pop(0)()

            def mk_norm(accf_, attn_u_, b_, hh_, qt_):
                def emit():
                    linv = lnvp.tile([128, QT], F32, tag="lnv", name="linv")
                    nc.vector.reciprocal_approx_fast(out=linv, in_=accf_)
                    nc.vector.tensor_mul(
                        attn_s[b_][:, hh_, qt_ * QT:(qt_ + 1) * QT],
                        attn_u_, linv)
                return emit

            def run_flight(b, hh2, tiles2, filler, pull_ns, skip_fill_groups=0):
                npair = len(hh2)
                prev = [None] * npair
                pvs = [None] * npair
                accs = [None] * npair
                gcnt = 0
                for qt in range(NQT):
                    flush_norm(keep=npair)
                    nkc = 4 * (qt + 1)
                    for X in range(npair):
                        pvs[X] = pvp.tile([128, QT], F32, tag="pv", name="pv")
                        accs[X] = accp.tile([128, QT], F16, tag="acc", name="acc")
                    for g in range(nkc // 2):
                        diag = g >= 2 * qt
                        rs = [max(0, 128 * (2 * g + s2 - 4 * qt)) for s2 in (0, 1)]
                        psels = [None] * npair
                        for X in range(npair):
                            qTp, kTp, _vp = tiles2[X]
                            acc = accs[X]
                            sc = ps2p.tile([128, 2, QT], F32, tag="ps2", name="sc")
                            for s2 in range(2):
                                kc = 2 * g + s2
                                nc.tensor.matmul(
                                    sc[:, s2, rs[s2]:],
                                    lhsT=kTp[:, kc * 128:(kc + 1) * 128],
                                    rhs=qTp[:, qt * QT + rs[s2]:(qt + 1) * QT],
                                    start=True, stop=True)
                            pt2 = ptp.tile([128, 2, QT], F16, tag="pt", name="pt2")
                            if diag:
                                for s2 in range(2):
                                    r = rs[s2]
                                    nc.scalar.activation(
                                        out=pt2[:, s2, r:], in_=sc[:, s2, r:],
                                        func=EXP, scale=SCALING)
                                psel = ptp.tile([128, 2, QT], F16, tag="pt",
                                                name="ptm")
                                for s2 in range(2):
                                    r = rs[s2]
                                    nc.vector.tensor_mul(
                                        psel[:, s2, r:], pt2[:, s2, r:],
                                        mskt[:, g - 2 * qt, s2, r:])
                            else:
                                nc.scalar.activation(out=pt2, in_=sc, func=EXP,
                                                     scale=SCALING)
                                psel = pt2
                            # softmax denominator accumulation (Vector, fp16 2x)
                            if g == 0:
                                if diag:  # qt == 0: rs == [0, 128]
                                    nc.vector.tensor_copy(out=acc, in_=psel[:, 0, :])
                                    nc.vector.tensor_add(
                                        acc[:, 128:], acc[:, 128:], psel[:, 1, 128:])
                                else:
                                    nc.vector.tensor_add(
                                        acc, psel[:, 0, :], psel[:, 1, :])
                            else:
                                for s2 in range(2):
                                    r = rs[s2]
                                    nc.vector.tensor_add(
                                        acc[:, r:], acc[:, r:], psel[:, s2, r:])
                            psels[X] = psel
                        gcnt += 1
                        if gcnt > skip_fill_groups:
                            filler.pull(pull_ns)
                        for X in range(npair):
                            if prev[X] is not None:
                                prev[X]()

                            def mk_pv(psel_, pv_, vp_, g_, rs_, nkc_):
                                def emit():
                                    for s2 in range(2):
                                        kc = 2 * g_ + s2
                                        nc.tensor.matmul(
                                            pv_[:, rs_[s2]:], lhsT=vp_[:, kc, :],
                                            rhs=psel_[:, s2, rs_[s2]:],
                                            start=(kc == 0), stop=(kc == nkc_ - 1))
                                return emit
                            prev[X] = mk_pv(psels[X], pvs[X], tiles2[X][2],
                                            g, rs, nkc)
                    for X in range(npair):
                        prev[X]()
                        prev[X] = None
                        # stage unnormalized attn to SBUF so the pv psum slot
                        # is released without waiting on the l-reduction chain
                        attn_u = aup.tile([128, QT], BF16, tag="au", name="attn_u")
                        nc.vector.tensor_copy(out=attn_u, in_=pvs[X])
                        accf = accfp.tile([128, QT], F32, tag="accf", name="accf")
                        nc.gpsimd.partition_all_reduce(
                            accf, accs[X], 128, ReduceOp.add)
                        pending_norm.append(mk_norm(accf, attn_u, b, hh2[X], qt))

            # ---------------- phase 1: qkv projections + rope ----------------
            def p1_stream(blocks, pools, cold_start=False):
                hp, wp, csp, rtp, stp = pools

                def load_w(j3, i, fine=False):
                    t_ = wp.tile([128, 8, FH], BF16, tag="wch", name="wch")
                    src = w[j3, i].rearrange("p (kc f) -> p kc f", kc=8)
                    if fine:
                        for kq in range(4):
                            nc.sync.dma_start(
                                out=t_[:, 2 * kq:2 * kq + 2, :],
                                in_=src[:, 2 * kq:2 * kq + 2, :])
                    else:
                        nc.sync.dma_start(out=t_, in_=src)
                    return t_

                for T in blocks:
                    first = cold_start and T == blocks[0]
                    hblk = []
                    first_w = []
                    for i in range(4):
                        t_ = hp.tile([128, 8, TBLK], BF16, tag="hblk", name="hblk")
                        src = hT[T, i].rearrange("p (kc t) -> p kc t", kc=8)
                        if first:
                            # cold start: interleave h and w(j3=0) tile loads so
                            # the first matmul's operands aren't queued behind
                            # 4MB of hidden-state packets
                            for kq in range(4):
                                nc.sync.dma_start(
                                    out=t_[:, 2 * kq:2 * kq + 2, :],
                                    in_=src[:, 2 * kq:2 * kq + 2, :])
                            first_w.append(load_w(0, i, fine=(i == 0)))
                        else:
                            nc.sync.dma_start(out=t_, in_=src)
                        hblk.append(t_)
                    csts = []
                    for tt in range(4):
                        cst = csp.tile([128, 2, HC, D], BF16, tag="cs", name="cst")
                        r0 = T * TBLK + tt * 128
                        nc.sync.dma_start(
                            out=cst,
                            in_=csn[r0:r0 + 128].rearrange("p c (h d) -> p c h d", h=HC))
                        csts.append(cst)
                    yield 0
                    for j3 in range(3):
                        if first and j3 == 0:
                            wch = first_w
                        else:
                            wch = [load_w(j3, i) for i in range(4)]
                        yield 0
                        for tt in range(4):
                            ps = psp.tile([128, HC, D], F32, tag="ps", name="ps")
                            for half in range(8):
                                i = half // 2
                                for kc in range(4 * (half % 2), 4 * (half % 2) + 4):
                                    nc.tensor.matmul(
                                        ps,
                                        lhsT=hblk[i][:, kc, tt * 128:(tt + 1) * 128],
                                        rhs=wch[i][:, kc, :],
                                        start=(half == 0 and kc == 0),
                                        stop=(half == 7 and kc == 7),
                                    )
                                yield 852
                            st = stp.tile([128, HC, D], BF16, tag="stage", name="st")
                            if j3 < 2:
                                cst = csts[tt]
                                half_d = D // 2
                                tr = rtp.tile([128, HC, D], F32, tag="rtmp", name="tr")
                                tcos = rtp.tile([128, HC, D], F32, tag="rtmp", name="tcos")
                                nc.vector.tensor_mul(
                                    tr[:, :, 0:half_d], ps[:, :, half_d:D],
                                    cst[:, 1, :, 0:half_d])
                                nc.vector.tensor_mul(
                                    tr[:, :, half_d:D], ps[:, :, 0:half_d],
                                    cst[:, 1, :, half_d:D])
                                nc.vector.tensor_mul(tcos, ps, cst[:, 0])
                                nc.vector.tensor_add(st, tr, tcos)
                            else:
                                nc.vector.tensor_copy(out=st, in_=ps)
                            r0 = tt * 128
                            nc.sync.dma_start(
                                out=scr[j3][T][r0:r0 + 128, :], in_=st)
                            yield 0

            # ---------------- phase 3: o_proj partial (transposed out) -------
            def p3_stream(tbs, pools, pso_srcs, evac="alt"):
                wop, ostp = pools
                cnt = 0
                for ob in range(NOB):
                    wot = wop.tile([128, HC, 128], BF16, tag="wo", name="wot")
                    nc.sync.dma_start(out=wot, in_=wo[ob])
                    yield 0
                    for tb in tbs:
                        pool, tag = pso_srcs[cnt % len(pso_srcs)]
                        pso = pool.tile([128, TBLK], F32, tag=tag, name="pso")
                        for kc in range(HC):
                            nc.tensor.matmul(
                                pso, lhsT=wot[:, kc, :],
                                rhs=attn_s[tb // 4][:, kc,
                                                    (tb % 4) * TBLK:(tb % 4 + 1) * TBLK],
                                start=(kc == 0), stop=(kc == HC - 1))
                        yield 852
                        ot = ostp.tile([128, TBLK], BF16, tag="ost", name="ot")
                        # keep scalar free for pair exps when evac="vector"
                        if evac == "vector" or cnt % 2 == 0:
                            nc.vector.tensor_copy(out=ot, in_=pso)
                        else:
                            nc.scalar.copy(out=ot, in_=pso)
                        nc.sync.dma_start(
                            out=outp[ob * 128:(ob + 1) * 128,
                                     tb * TBLK:(tb + 1) * TBLK], in_=ot)
                        cnt += 1
                        yield 0

            # ---------------- schedule -------------------------------------
            with ExitStack() as p1ctx:
                p1pools = (
                    p1ctx.enter_context(tc.tile_pool(name="hblk", bufs=6)),
                    p1ctx.enter_context(tc.tile_pool(name="wch", bufs=6)),
                    p1ctx.enter_context(tc.tile_pool(name="cs", bufs=5)),
                    p1ctx.enter_context(tc.tile_pool(name="rtmp", bufs=2)),
                    p1ctx.enter_context(tc.tile_pool(name="stage", bufs=4)),
                )
                Filler([p1_stream(range(4), p1pools, cold_start=True)]).drain()
                fb = Filler([p1_stream(range(4, NTB), p1pools)])
                t00 = prefetch_flight(0, (0,), pairp)[0]
                t01 = prefetch_flight(0, (1,), pairp)[0]
                # no fill in the first 6 groups: p1B's first DMAs are ~15us
                # out, a stalled fill unit would block the in-order PE
                run_flight(0, (0,), (t00,), fb, pull_ns=700, skip_fill_groups=6)
                fb.pull(5000)
                t02 = prefetch_flight(0, (2,), pairp)[0]
                run_flight(0, (1,), (t01,), fb, pull_ns=700)
                fb.pull(5000)
                t03 = prefetch_flight(0, (3,), pairp)[0]
                run_flight(0, (2,), (t02,), fb, pull_ns=700)
                run_flight(0, (3,), (t03,), fb, pull_ns=700)
                flush_norm(keep=0)
                t10 = prefetch_flight(1, (0,), pairp)[0]
                fb.drain()

            with ExitStack() as p3ctx:
                p3pools = (
                    p3ctx.enter_context(tc.tile_pool(name="wop", bufs=3)),
                    p3ctx.enter_context(tc.tile_pool(name="ost", bufs=6)),
                )
                fc = Filler([p3_stream(range(4), p3pools, [(psp, "ps")],
                                       evac="alt")])
                fc.pull(12000)
                t11 = prefetch_flight(1, (1,), pairp)[0]
                # 2 fill units per group in b1: covers mid-pair chain bubbles;
                # evac (alt) stays ~50% loaded on scalar and vector
                run_flight(1, (0,), (t10,), fc, pull_ns=1550)
                t12 = prefetch_flight(1, (2,), pairp)[0]
                run_flight(1, (1,), (t11,), fc, pull_ns=1550)
                t13 = prefetch_flight(1, (3,), pairp)[0]
                run_flight(1, (2,), (t12,), fc, pull_ns=1550)
                run_flight(1, (3,), (t13,), fc, pull_ns=1550)
                flush_norm(keep=0)
                fc.drain()
                Filler([p3_stream(range(4, NTB), p3pools,
                                  [(psp, "ps"), (pvp, "pv"),
                                   (ps2p, "ps2")])]).drain()

    nc.compile()
    return nc


_NC_CACHE = {}


def get_nc():
    if "nc" not in _NC_CACHE:
        _NC_CACHE["nc"] = build_nc()
    return _NC_CACHE["nc"]


def prep_in_maps(positions, hidden_states, W_qkv, W_o):
    """Host-side sharding + layout prep. Returns per-core input maps."""
    bf16 = ml_dtypes.bfloat16
    hid = np.asarray(hidden_states, np.float32).reshape(TOK, HIDDEN)
    # hT[T, i, p, kc, t] = hid[T*512+t, i*1024+kc*128+p]
    hT = np.ascontiguousarray(
        hid.reshape(NTB, TBLK, 4, 8, 128).transpose(0, 2, 4, 3, 1)
    ).reshape(NTB, 4, 128, 8 * TBLK).astype(bf16)

    pos = np.asarray(positions).reshape(TOK).astype(np.float32)
    half = D // 2
    inv = ROPE_BASE ** (-np.arange(half, dtype=np.float32) / half)
    ang = pos[:, None] * inv[None, :]                      # [TOK, 64]
    cos = np.cos(ang)
    sin = np.sin(ang)
    cos128 = np.concatenate([cos, cos], axis=1)            # [TOK, 128]
    sin128 = np.concatenate([-sin, sin], axis=1)
    csn = np.empty((TOK, 2, FH), np.float32)
    csn[:, 0, :] = np.tile(cos128, HC)
    csn[:, 1, :] = np.tile(sin128, HC)
    csn = csn.astype(bf16)

    kk = np.arange(128)[:, None]
    qq = np.arange(QT)[None, :]
    msk = np.stack([(qq >= kk + o * 128) for o in range(4)], axis=1)
    msk = msk.reshape(128, 2, 2, QT).astype(np.float16)     # [128, 2, 2, 512]

    Wq = np.asarray(W_qkv, np.float32)
    Wo = np.asarray(W_o, np.float32)
    in_maps = []
    for c in range(N_CORES):
        wc = np.concatenate(
            [Wq[:, q0 * HIDDEN + c * FH: q0 * HIDDEN + (c + 1) * FH]
             for q0 in range(3)], axis=1)                   # [4096, 1536]
        # w[j3, i, p, kc, f] = wc[i*1024+kc*128+p, j3*512+f]
        wcp = np.ascontiguousarray(
            wc.reshape(4, 8, 128, 3, FH).transpose(3, 0, 2, 1, 4)
        ).reshape(3, 4, 128, 8 * FH).astype(bf16)
        woc = Wo[c * FH:(c + 1) * FH, :]                    # [512, 4096]
        # wo[ob, p, kc, o] = woc[kc*128+p, ob*128+o]
        wop = np.ascontiguousarray(
            woc.reshape(HC, 128, NOB, 128).transpose(2, 1, 0, 3)
        ).astype(bf16)
        in_maps.append({"hT": hT, "w": wcp, "wo": wop, "csn": csn, "msk": msk})
    return in_maps


def combine_outputs(outps):
    """Sum per-core bf16 o_partial^T [HIDDEN, TOK] and untranspose."""
    out = np.zeros((HIDDEN, TOK), np.float32)
    for o in outps:
        out += np.asarray(o).astype(np.float32)
    return np.ascontiguousarray(out.T).astype(np.float32).reshape(B, S, HIDDEN)


def kernel(positions, hidden_states, W_qkv, W_o):
    nc = get_nc()
    in_maps = prep_in_maps(positions, hidden_states, W_qkv, W_o)
    res = run_bass_kernel_spmd(nc, in_maps, list(range(N_CORES)))
    return combine_outputs([res.results[c]["outp"] for c in range(N_CORES)])


# revision 38
# speedup vs baseline: 1.0547x; 1.0413x over previous
"""Llama attention layer (B=2, S=2048, H=4096, 32 heads, fp32 io) on 8 trn2 cores.

Sharding: tensor-parallel over heads. Each core owns 4 heads: W_qkv column
shard [4096, 3*512] (bf16), W_o row shard [512, 4096] (bf16). Each core
computes qkv proj + RoPE + causal attention for its heads + its o_proj
partial; the host sums the 8 partials (the "all-reduce") and untransposes
the output (kernel emits o_partial^T in bf16).

v3 vs v2 (overlap restructure, from trace: PE busy was 88.3%, with the
b1-pairs+p3 region 70us over floor from shallow p3 psum groups whose
evacuations queued behind pair work in the strict-FIFO engines):
  - attention pairs run as 2-wide interleaved "flights": pair A's exp/mask
    chain latency is hidden behind pair B's score/PV matmuls, so the PE
    self-fills during attention and the p3/p1 filler is only a top-up.
  - pair prefetches are emitted in 512-token chunks, so transposes fire
    progressively as the producing phase-1 blocks complete (pair starts
    and batch boundaries no longer expose the ~15us transpose latency).
  - softmax row-sums accumulate in fp16 (2x DVE 16-bit mode); the gpsimd
    partition-reduce upcasts to fp32 for the reciprocal.
  - o_proj partials are written bf16 (halves the 67MB output DMA).
  - p3 tail rotates its psum tiles across 3 pools (6 banks) so matmul
    groups never wait on evacuation.
  - first hblk/wch DMAs split per-kc so the first matmul starts sooner.
"""

import numpy as np
import ml_dtypes

import concourse.bass as bass
import concourse.tile as tile
from concourse import bacc, mybir
from concourse.bass_isa import ReduceOp
from concourse.bass_utils import run_bass_kernel_spmd

# ---- problem constants (hardcoded per contract) ----
HIDDEN = 4096
NH = 32
D = 128
B = 2
S = 2048
TOK = B * S            # 4096 tokens
N_CORES = 8
HC = NH // N_CORES     # 4 heads per core
FH = HC * D            # 512 features per core for each of q/k/v
SCALING = D ** -0.5
ROPE_BASE = 10000.0

BF16 = mybir.dt.bfloat16
F16 = mybir.dt.float16
F32 = mybir.dt.float32

TBLK = 512             # tokens per phase-1 block
NTB = TOK // TBLK      # 8
QT = 512               # q columns per phase-2 tile
NQT = S // QT          # 4
NKC = S // 128         # 16 k chunks per sequence
NOB = HIDDEN // 128    # 32 output-column chunks in phase 3
EXP = mybir.ActivationFunctionType.Exp


class Filler:
    """Pulls emission units (generators yielding ~tensor-ns) on demand."""

    def __init__(self, gens):
        self.gens = list(gens)

    def pull(self, ns):
        while ns > 0 and self.gens:
            try:
                ns -= next(self.gens[0])
            except StopIteration:
                self.gens.pop(0)

    def drain(self):
        self.pull(float("inf"))


def build_nc():
    nc = bacc.Bacc("TRN2", target_bir_lowering=False, debug=False,
                   num_devices=N_CORES)
    hT = nc.dram_tensor("hT", [NTB, 4, 128, 8 * TBLK], BF16, kind="ExternalInput").ap()
    w = nc.dram_tensor("w", [3, 4, 128, 8 * FH], BF16, kind="ExternalInput").ap()
    wo = nc.dram_tensor("wo", [NOB, 128, HC, 128], BF16, kind="ExternalInput").ap()
    csn = nc.dram_tensor("csn", [TOK, 2, FH], BF16, kind="ExternalInput").ap()
    msk = nc.dram_tensor("msk", [128, 2, 2, QT], F16, kind="ExternalInput").ap()
    outp = nc.dram_tensor("outp", [HIDDEN, TOK], BF16, kind="ExternalOutput").ap()

    with tile.TileContext(nc) as tc:
        from contextlib import ExitStack
        with ExitStack() as ctx:
            # PSUM: ps 2 banks + ps2 4 banks + pv 2 banks = 8
            psp = ctx.enter_context(tc.tile_pool(name="ps", bufs=2, space="PSUM"))
            ps2p = ctx.enter_context(tc.tile_pool(name="ps2", bufs=2, space="PSUM"))
            pvp = ctx.enter_context(tc.tile_pool(name="pv", bufs=2, space="PSUM"))
            gsb = ctx.enter_context(tc.tile_pool(name="gsb", bufs=1))
            pairp = ctx.enter_context(tc.tile_pool(name="pair", bufs=6))
            ptp = ctx.enter_context(tc.tile_pool(name="pt", bufs=6))
            accp = ctx.enter_context(tc.tile_pool(name="acc", bufs=4))
            accfp = ctx.enter_context(tc.tile_pool(name="accf", bufs=4))
            lnvp = ctx.enter_context(tc.tile_pool(name="lnv", bufs=2))
            aup = ctx.enter_context(tc.tile_pool(name="au", bufs=4))
            dscr = ctx.enter_context(tc.tile_pool(name="dscr", bufs=1, space="DRAM"))

            # per-block scratch tiles: DRAM deps are tile-granular, so pair
            # transposes of block c can fire as soon as block c's projection
            # lands instead of waiting for the whole batch
            scr = [[dscr.tile([TBLK, FH], BF16, tag=f"s{j3}_{T}",
                              name=f"scr{j3}_{T}") for T in range(NTB)]
                   for j3 in range(3)]

            attn_s = [gsb.tile([128, HC, S], BF16, tag="attn0", name="attn0"),
                      gsb.tile([128, HC, S], BF16, tag="attn1", name="attn1")]
            mskt = gsb.tile([128, 2, 2, QT], F16, tag="msk", name="mskt")
            nc.sync.dma_start(out=mskt, in_=msk)

            # ---------------- phase 2: interleaved pair flights ---------------
            def prefetch_flight(b, hhs, pool):
                # chunked so each transpose fires as its phase-1 block lands;
                # issued from the SCALAR queue: the sync queue is an in-order
                # pipe paced by phase-1's pool-slot waits, so transposes
                # issued there can't start until the whole drain clears (and
                # they'd head-of-line block later fill DMAs). Scalar is idle
                # outside the exp bursts.
                tiles = []
                for hh in hhs:
                    qTp = pool.tile([128, S], BF16, tag="pair", name="qTp")
                    kTp = pool.tile([128, S], BF16, tag="pair", name="kTp")
                    vp = pool.tile([128, NKC, D], BF16, tag="pair", name="vp")
                    tiles.append((qTp, kTp, vp))
                for c in range(4):
                    blk = b * 4 + c
                    for hh, (qTp, kTp, vp) in zip(hhs, tiles):
                        cols = slice(hh * D, (hh + 1) * D)
                        nc.sync.dma_start_transpose(
                            out=qTp[:, c * TBLK:(c + 1) * TBLK],
                            in_=scr[0][blk][:, cols])
                        nc.sync.dma_start_transpose(
                            out=kTp[:, c * TBLK:(c + 1) * TBLK],
                            in_=scr[1][blk][:, cols])
                        nc.sync.dma_start(
                            out=vp[:, 4 * c:4 * (c + 1), :],
                            in_=scr[2][blk][:, cols].rearrange(
                                "(kc p) d -> p kc d", p=128))
                return tiles

            pending_norm = []

            def flush_norm(keep):
                # deferred reciprocal+normalize: by flush time the gpsimd
                # all_reduce is long done, so Vector never blocks on it
                while len(pending_norm) > keep:
                    pending_norm.pop(0)()

            def mk_norm(accf_, attn_u_, b_, hh_, qt_):
                def emit():
                    linv = lnvp.tile([128, QT], F32, tag="lnv", name="linv")
                    nc.vector.reciprocal_approx_fast(out=linv, in_=accf_)
                    nc.vector.tensor_mul(
                        attn_s[b_][:, hh_, qt_ * QT:(qt_ + 1) * QT],
                        attn_u_, linv)
                return emit

            def run_flight(b, hh2, tiles2, filler, pull_ns, skip_fill_groups=0):
                npair = len(hh2)
                prev = [None] * npair
                pvs = [None] * npair
                accs = [None] * npair
                gcnt = 0
                for qt in range(NQT):
                    flush_norm(keep=npair)
                    nkc = 4 * (qt + 1)
                    for X in range(npair):
                        pvs[X] = pvp.tile([128, QT], F32, tag="pv", name="pv")
                        accs[X] = accp.tile([128, QT], F16, tag="acc", name="acc")
                    for g in range(nkc // 2):
                        diag = g >= 2 * qt
                        rs = [max(0, 128 * (2 * g + s2 - 4 * qt)) for s2 in (0, 1)]
                        psels = [None] * npair
                        for X in range(npair):
                            qTp, kTp, _vp = tiles2[X]
                            acc = accs[X]
                            sc = ps2p.tile([128, 2, QT], F32, tag="ps2", name="sc")
                            for s2 in range(2):
                                kc = 2 * g + s2
                                nc.tensor.matmul(
                                    sc[:, s2, rs[s2]:],
                                    lhsT=kTp[:, kc * 128:(kc + 1) * 128],
                                    rhs=qTp[:, qt * QT + rs[s2]:(qt + 1) * QT],
                                    start=True, stop=True)
                            pt2 = ptp.tile([128, 2, QT], F16, tag="pt", name="pt2")
                            if diag:
                                for s2 in range(2):
                                    r = rs[s2]
                                    nc.scalar.activation(
                                        out=pt2[:, s2, r:], in_=sc[:, s2, r:],
                                        func=EXP, scale=SCALING)
                                psel = ptp.tile([128, 2, QT], F16, tag="pt",
                                                name="ptm")
                                for s2 in range(2):
                                    r = rs[s2]
                                    nc.vector.tensor_mul(
                                        psel[:, s2, r:], pt2[:, s2, r:],
                                        mskt[:, g - 2 * qt, s2, r:])
                            else:
                                nc.scalar.activation(out=pt2, in_=sc, func=EXP,
                                                     scale=SCALING)
                                psel = pt2
                            # softmax denominator accumulation (Vector, fp16 2x)
                            if g == 0:
                                if diag:  # qt == 0: rs == [0, 128]
                                    nc.vector.tensor_copy(out=acc, in_=psel[:, 0, :])
                                    nc.vector.tensor_add(
                                        acc[:, 128:], acc[:, 128:], psel[:, 1, 128:])
                                else:
                                    nc.vector.tensor_add(
                                        acc, psel[:, 0, :], psel[:, 1, :])
                            else:
                                for s2 in range(2):
                                    r = rs[s2]
                                    nc.vector.tensor_add(
                                        acc[:, r:], acc[:, r:], psel[:, s2, r:])
                            psels[X] = psel
                        gcnt += 1
                        if gcnt > skip_fill_groups:
                            filler.pull(pull_ns)
                        for X in range(npair):
                            if prev[X] is not None:
                                prev[X]()

                            def mk_pv(psel_, pv_, vp_, g_, rs_, nkc_):
                                def emit():
                                    for s2 in range(2):
                                        kc = 2 * g_ + s2
                                        nc.tensor.matmul(
                                            pv_[:, rs_[s2]:], lhsT=vp_[:, kc, :],
                                            rhs=psel_[:, s2, rs_[s2]:],
                                            start=(kc == 0), stop=(kc == nkc_ - 1))
                                return emit
                            prev[X] = mk_pv(psels[X], pvs[X], tiles2[X][2],
                                            g, rs, nkc)
                    for X in range(npair):
                        prev[X]()
                        prev[X] = None
                        # stage unnormalized attn to SBUF so the pv psum slot
                        # is released without waiting on the l-reduction chain
                        attn_u = aup.tile([128, QT], BF16, tag="au", name="attn_u")
                        nc.vector.tensor_copy(out=attn_u, in_=pvs[X])
                        accf = accfp.tile([128, QT], F32, tag="accf", name="accf")
                        nc.gpsimd.partition_all_reduce(
                            accf, accs[X], 128, ReduceOp.add)
                        pending_norm.append(mk_norm(accf, attn_u, b, hh2[X], qt))

            # ---------------- phase 1: qkv projections + rope ----------------
            def p1_stream(blocks, pools, cold_start=False):
                hp, wp, csp, rtp, stp = pools

                def load_w(j3, i, fine=False):
                    t_ = wp.tile([128, 8, FH], BF16, tag="wch", name="wch")
                    src = w[j3, i].rearrange("p (kc f) -> p kc f", kc=8)
                    if fine:
                        for kq in range(4):
                            nc.sync.dma_start(
                                out=t_[:, 2 * kq:2 * kq + 2, :],
                                in_=src[:, 2 * kq:2 * kq + 2, :])
                    else:
                        nc.sync.dma_start(out=t_, in_=src)
                    return t_

                for T in blocks:
                    first = cold_start and T == blocks[0]
                    hblk = []
                    first_w = []
                    for i in range(4):
                        t_ = hp.tile([128, 8, TBLK], BF16, tag="hblk", name="hblk")
                        src = hT[T, i].rearrange("p (kc t) -> p kc t", kc=8)
                        if first:
                            # cold start: interleave h and w(j3=0) tile loads so
                            # the first matmul's operands aren't queued behind
                            # 4MB of hidden-state packets
                            for kq in range(4):
                                nc.sync.dma_start(
                                    out=t_[:, 2 * kq:2 * kq + 2, :],
                                    in_=src[:, 2 * kq:2 * kq + 2, :])
                            first_w.append(load_w(0, i, fine=(i == 0)))
                        else:
                            nc.sync.dma_start(out=t_, in_=src)
                        hblk.append(t_)
                    csts = []
                    for tt in range(4):
                        cst = csp.tile([128, 2, HC, D], BF16, tag="cs", name="cst")
                        r0 = T * TBLK + tt * 128
                        nc.sync.dma_start(
                            out=cst,
                            in_=csn[r0:r0 + 128].rearrange("p c (h d) -> p c h d", h=HC))
                        csts.append(cst)
                    yield 0
                    for j3 in range(3):
                        if first and j3 == 0:
                            wch = first_w
                        else:
                            wch = [load_w(j3, i) for i in range(4)]
                        yield 0
                        for tt in range(4):
                            ps = psp.tile([128, HC, D], F32, tag="ps", name="ps")
                            for half in range(8):
                                i = half // 2
                                for kc in range(4 * (half % 2), 4 * (half % 2) + 4):
                                    nc.tensor.matmul(
                                        ps,
                                        lhsT=hblk[i][:, kc, tt * 128:(tt + 1) * 128],
                                        rhs=wch[i][:, kc, :],
                                        start=(half == 0 and kc == 0),
                                        stop=(half == 7 and kc == 7),
                                    )
                                yield 852
                            st = stp.tile([128, HC, D], BF16, tag="stage", name="st")
                            if j3 < 2:
                                cst = csts[tt]
                                half_d = D // 2
                                tr = rtp.tile([128, HC, D], F32, tag="rtmp", name="tr")
                                tcos = rtp.tile([128, HC, D], F32, tag="rtmp", name="tcos")
                                nc.vector.tensor_mul(
                                    tr[:, :, 0:half_d], ps[:, :, half_d:D],
                                    cst[:, 1, :, 0:half_d])
                                nc.vector.tensor_mul(
                                    tr[:, :, half_d:D], ps[:, :, 0:half_d],
                                    cst[:, 1, :, half_d:D])
                                nc.vector.tensor_mul(tcos, ps, cst[:, 0])
                                nc.vector.tensor_add(st, tr, tcos)
                            else:
                                nc.vector.tensor_copy(out=st, in_=ps)
                            r0 = tt * 128
                            nc.sync.dma_start(
                                out=scr[j3][T][r0:r0 + 128, :], in_=st)
                            yield 0

            # ---------------- phase 3: o_proj partial (transposed out) -------
            def p3_stream(tbs, pools, pso_srcs, evac="alt"):
                wop, ostp = pools
                cnt = 0
                for ob in range(NOB):
                    wot = wop.tile([128, HC, 128], BF16, tag="wo", name="wot")
                    nc.sync.dma_start(out=wot, in_=wo[ob])
                    yield 0
                    for tb in tbs:
                        pool, tag = pso_srcs[cnt % len(pso_srcs)]
                        pso = pool.tile([128, TBLK], F32, tag=tag, name="pso")
                        for kc in range(HC):
                            nc.tensor.matmul(
                                pso, lhsT=wot[:, kc, :],
                                rhs=attn_s[tb // 4][:, kc,
                                                    (tb % 4) * TBLK:(tb % 4 + 1) * TBLK],
                                start=(kc == 0), stop=(kc == HC - 1))
                        yield 852
                        ot = ostp.tile([128, TBLK], BF16, tag="ost", name="ot")
                        # keep scalar free for pair exps when evac="vector"
                        if evac == "vector" or cnt % 2 == 0:
                            nc.vector.tensor_copy(out=ot, in_=pso)
                        else:
                            nc.scalar.copy(out=ot, in_=pso)
                        nc.sync.dma_start(
                            out=outp[ob * 128:(ob + 1) * 128,
                                     tb * TBLK:(tb + 1) * TBLK], in_=ot)
                        cnt += 1
                        yield 0

            # ---------------- schedule -------------------------------------
            with ExitStack() as p1ctx:
                p1pools = (
                    p1ctx.enter_context(tc.tile_pool(name="hblk", bufs=6)),
                    p1ctx.enter_context(tc.tile_pool(name="wch", bufs=6)),
                    p1ctx.enter_context(tc.tile_pool(name="cs", bufs=5)),
                    p1ctx.enter_context(tc.tile_pool(name="rtmp", bufs=2)),
                    p1ctx.enter_context(tc.tile_pool(name="stage", bufs=4)),
                )
                Filler([p1_stream(range(4), p1pools, cold_start=True)]).drain()
                fb = Filler([p1_stream(range(4, NTB), p1pools)])
                t00 = prefetch_flight(0, (0,), pairp)[0]
                t01 = prefetch_flight(0, (1,), pairp)[0]
                # no fill in the first 6 groups: p1B's first DMAs are ~15us
                # out, a stalled fill unit would block the in-order PE
                run_flight(0, (0,), (t00,), fb, pull_ns=700, skip_fill_groups=6)
                t02 = prefetch_flight(0, (2,), pairp)[0]
                run_flight(0, (1,), (t01,), fb, pull_ns=700)
                t03 = prefetch_flight(0, (3,), pairp)[0]
                run_flight(0, (2,), (t02,), fb, pull_ns=700)
                run_flight(0, (3,), (t03,), fb, pull_ns=700)
                flush_norm(keep=0)
                t10 = prefetch_flight(1, (0,), pairp)[0]
                fb.drain()

            with ExitStack() as p3ctx:
                p3pools = (
                    p3ctx.enter_context(tc.tile_pool(name="wop", bufs=3)),
                    p3ctx.enter_context(tc.tile_pool(name="ost", bufs=6)),
                )
                fc = Filler([p3_stream(range(4), p3pools, [(psp, "ps")],
                                       evac="alt")])
                fc.pull(12000)
                t11 = prefetch_flight(1, (1,), pairp)[0]
                run_flight(1, (0,), (t10,), fc, pull_ns=700)
                t12 = prefetch_flight(1, (2,), pairp)[0]
                run_flight(1, (1,), (t11,), fc, pull_ns=700)
                t13 = prefetch_flight(1, (3,), pairp)[0]
                run_flight(1, (2,), (t12,), fc, pull_ns=700)
                run_flight(1, (3,), (t13,), fc, pull_ns=700)
                flush_norm(keep=0)
                fc.drain()
                Filler([p3_stream(range(4, NTB), p3pools,
                                  [(psp, "ps"), (pvp, "pv"),
                                   (ps2p, "ps2")])]).drain()

    nc.compile()
    return nc


_NC_CACHE = {}


def get_nc():
    if "nc" not in _NC_CACHE:
        _NC_CACHE["nc"] = build_nc()
    return _NC_CACHE["nc"]


def prep_in_maps(positions, hidden_states, W_qkv, W_o):
    """Host-side sharding + layout prep. Returns per-core input maps."""
    bf16 = ml_dtypes.bfloat16
    hid = np.asarray(hidden_states, np.float32).reshape(TOK, HIDDEN)
    # hT[T, i, p, kc, t] = hid[T*512+t, i*1024+kc*128+p]
    hT = np.ascontiguousarray(
        hid.reshape(NTB, TBLK, 4, 8, 128).transpose(0, 2, 4, 3, 1)
    ).reshape(NTB, 4, 128, 8 * TBLK).astype(bf16)

    pos = np.asarray(positions).reshape(TOK).astype(np.float32)
    half = D // 2
    inv = ROPE_BASE ** (-np.arange(half, dtype=np.float32) / half)
    ang = pos[:, None] * inv[None, :]                      # [TOK, 64]
    cos = np.cos(ang)
    sin = np.sin(ang)
    cos128 = np.concatenate([cos, cos], axis=1)            # [TOK, 128]
    sin128 = np.concatenate([-sin, sin], axis=1)
    csn = np.empty((TOK, 2, FH), np.float32)
    csn[:, 0, :] = np.tile(cos128, HC)
    csn[:, 1, :] = np.tile(sin128, HC)
    csn = csn.astype(bf16)

    kk = np.arange(128)[:, None]
    qq = np.arange(QT)[None, :]
    msk = np.stack([(qq >= kk + o * 128) for o in range(4)], axis=1)
    msk = msk.reshape(128, 2, 2, QT).astype(np.float16)     # [128, 2, 2, 512]

    Wq = np.asarray(W_qkv, np.float32)
    Wo = np.asarray(W_o, np.float32)
    in_maps = []
    for c in range(N_CORES):
        wc = np.concatenate(
            [Wq[:, q0 * HIDDEN + c * FH: q0 * HIDDEN + (c + 1) * FH]
             for q0 in range(3)], axis=1)                   # [4096, 1536]
        # w[j3, i, p, kc, f] = wc[i*1024+kc*128+p, j3*512+f]
        wcp = np.ascontiguousarray(
            wc.reshape(4, 8, 128, 3, FH).transpose(3, 0, 2, 1, 4)
        ).reshape(3, 4, 128, 8 * FH).astype(bf16)
        woc = Wo[c * FH:(c + 1) * FH, :]                    # [512, 4096]
        # wo[ob, p, kc, o] = woc[kc*128+p, ob*128+o]
        wop = np.ascontiguousarray(
            woc.reshape(HC, 128, NOB, 128).transpose(2, 1, 0, 3)
        ).astype(bf16)
        in_maps.append({"hT": hT, "w": wcp, "wo": wop, "csn": csn, "msk": msk})
    return in_maps


def combine_outputs(outps):
    """Sum per-core bf16 o_partial^T [HIDDEN, TOK] and untranspose."""
    out = np.zeros((HIDDEN, TOK), np.float32)
    for o in outps:
        out += np.asarray(o).astype(np.float32)
    return np.ascontiguousarray(out.T).astype(np.float32).reshape(B, S, HIDDEN)


def kernel(positions, hidden_states, W_qkv, W_o):
    nc = get_nc()
    in_maps = prep_in_maps(positions, hidden_states, W_qkv, W_o)
    res = run_bass_kernel_spmd(nc, in_maps, list(range(N_CORES)))
    return combine_outputs([res.results[c]["outp"] for c in range(N_CORES)])
